# revision 57
# baseline (speedup 1.0000x reference)
"""DiT block kernel for Trainium2 (Bass/Tile), 8-core data parallel.

Shapes (hardcoded from the problem spec):
  x: (8, 1024, 1152), t_emb: (8, 1152)
  w_qkv (1152, 3456), w_proj (1152, 1152), w_fc1 (1152, 4608),
  w_fc2 (4608, 1152), w_ada (1152, 6912) + biases.

Strategy: batch-parallel across 8 cores (one batch element each, no
collectives). Activations live transposed [D on partitions, tokens free].
All large GEMMs run in fp8e4 with DoubleRow packing (contraction pairs
of 128-partition tiles -> 2x PE throughput); weights are scaled x256
into fp8 range on-chip, outputs rescaled 1/256 in the PSUM-drain
activation. Attention scores use a [36,2] DoubleRow split of the 72-dim
head contraction; exp is computed as exp(s*isc - ln16) so softmax
numerators stay inside fp8 range (the denominator ratio is
scale-invariant, and the v-bias rides inside v so softmax-avg adds it
for free). ada projection and LN-stat matmuls stream f32 operands
bitcast to f32r (full PE rate at N>=256, zero conversion cost).
All big weight matrices stream through one long-lived flat
[128, 3456]-f32 pool so DMA prefetch is never write-after-read blocked
by phase-local pools; fp8 conversion happens on DVE/GPSIMD (the
activation engine is the attention-phase bottleneck via exp).
attn head->tile regrouping goes through a DRAM round trip.
"""

import math
import os
import threading
from contextlib import ExitStack

import numpy as np

import concourse.bass as bass
import concourse.mybir as mybir
import concourse.tile as tile
from concourse import bacc
from concourse.bass_utils import run_bass_kernel_spmd
from concourse.masks import make_identity

F32 = mybir.dt.float32
F32R = mybir.dt.float32r
BF16 = mybir.dt.bfloat16
F8 = mybir.dt.float8e4
AF = mybir.ActivationFunctionType
ALU = mybir.AluOpType
DRM = mybir.MatmulPerfMode.DoubleRow

NCORES = 8
D = 1152
NT = 1024          # tokens per core (batch element)
KT = D // 128      # 9 partition-tiles of D
H = 16
HD = 72
HID = 4 * D        # 4608
MQK = (2 * D) // 128   # 18 output tiles for q,k
MH = HID // 128        # 36
EPS = 1e-6
ISC = 1.0 / float(np.sqrt(HD))
WS = 256.0         # weight scale into fp8
OS = 1.0 / WS
LN16 = math.log(16.0)


def _r(ap):
    return ap.bitcast(F32R)


def _head_segs(d0, n):
    """Split logical rows [d0, d0+n) of a [*,128]-tiled stacked tensor into
    (ktile, part0, length, dst_offset) segments within 128-partition tiles."""
    segs = []
    off = 0
    while n > 0:
        kt_i, p0 = divmod(d0, 128)
        ln = min(n, 128 - p0)
        segs.append((kt_i, p0, ln, off))
        d0 += ln
        off += ln
        n -= ln
    return segs


def _col_head_segs(c0, n):
    """Split v-feature columns [c0, c0+n) into (head, col0, len, src_off)
    pieces."""
    segs = []
    off = 0
    while n > 0:
        h, j0 = divmod(c0, HD)
        ln = min(n, HD - j0)
        segs.append((h, j0, ln, off))
        c0 += ln
        off += ln
        n -= ln
    return segs


def _build_program():
    nc = bacc.Bacc(
        "TRN2", target_bir_lowering=False, debug=False, enable_asserts=False
    )
    ins = {}
    ins["x"] = nc.dram_tensor("x", [NT, D], F32, kind="ExternalInput").ap()
    ins["t_emb"] = nc.dram_tensor("t_emb", [D], F32, kind="ExternalInput").ap()
    for name, shape in [
        ("w_qkv", [D, 3 * D]), ("b_qkv", [3 * D]),
        ("w_proj", [D, D]), ("b_proj", [D]),
        ("w_fc1", [D, HID]), ("b_fc1", [HID]),
        ("w_fc2", [HID, D]), ("b_fc2", [D]),
        ("w_ada", [D, 6 * D]), ("b_ada", [6 * D]),
    ]:
        ins[name] = nc.dram_tensor(name, shape, F32, kind="ExternalInput").ap()
    out_dram = nc.dram_tensor("out", [NT, D], F32, kind="ExternalOutput").ap()

    with tile.TileContext(nc) as tc:
        _body(tc, ins, out_dram)
    nc.compile()
    return nc


def _conv8(nc, i, dst, src):
    """f32 -> fp8e4 with x256 scale, DVE/GPSIMD round-robined by index i
    (the activation engine is kept free for exp/gelu/modulate)."""
    if i % 2 == 0:
        nc.vector.tensor_scalar_mul(dst, src, WS)
    else:
        nc.gpsimd.tensor_scalar_mul(dst, src, WS)


def _dr_group(nc, out, w8, msl, act, nsl):
    """out = (w8[:,:,msl]*256).T @ act[:,:,nsl] over KT=9 k-tiles:
    4 DoubleRow pairs + 1 plain fp8 matmul."""
    for j in range(4):
        nc.tensor.matmul(
            out, w8[:, 2 * j:2 * j + 2, msl], act[:, 2 * j:2 * j + 2, nsl],
            start=(j == 0), stop=False, perf_mode=DRM,
            skip_group_check=True,
        )
    nc.tensor.matmul(
        out, w8[:, 8, msl], act[:, 8, nsl], start=False, stop=True,
        skip_group_check=True,
    )


def _ln_stats_and_modulate(tc, nc, src, dst, ada_pp, shift_c, scale_c,
                           ones_r, eps_sb, pst, pln, ps_st):
    """dst[:,k,:] = fp8(((src-mean)*rstd) * ada[scale_c] + ada[shift_c]).
    mean/var over the partition (D) axis per token via ones-vector f32r
    matmuls. Stats for both 512-token halves first, then applies; the
    normalize sub/mul pairs are split DVE/GPSIMD to shorten the chain."""
    ps_x, ps_q, st = {}, {}, {}
    for n in range(2):
        nsl = slice(n * 512, (n + 1) * 512)
        ps_x[n] = ps_st.tile([1, 512], F32, tag="st", name=f"psx{n}")
        ps_q[n] = ps_st.tile([1, 512], F32, tag="st", name=f"psq{n}")
        for k in range(KT):
            sq = pln.tile([128, 512], F32R, tag="sq", bufs=3, name="sq")
            sqe = nc.gpsimd if k % 3 == 1 else nc.vector
            sqe.tensor_mul(sq[:, :], src[:, k, nsl], src[:, k, nsl])
            nc.tensor.matmul(
                ps_x[n][:, :], ones_r[:, :], src[:, k, nsl],
                start=(k == 0), stop=(k == KT - 1), skip_group_check=True,
            )
            nc.tensor.matmul(
                ps_q[n][:, :], ones_r[:, :], sq[:, :],
                start=(k == 0), stop=(k == KT - 1), skip_group_check=True,
            )
    for n in range(2):
        # rows: 0 = mean, 1 = E[x^2] -> rstd
        st[n] = pst.tile([1, 2, 512], F32, tag="lnst", bufs=2, name=f"st{n}")
        nc.vector.tensor_scalar_mul(st[n][:, 0, :], ps_x[n][:, :], 1.0 / D)
        work = pst.tile([1, 512], F32, tag="lnwork", bufs=2, name="work")
        nc.vector.tensor_mul(work[:, :], st[n][:, 0, :], st[n][:, 0, :])
        nc.vector.scalar_tensor_tensor(
            st[n][:, 1, :], ps_q[n][:, :], 1.0 / D, work[:, :],
            ALU.mult, ALU.subtract,
        )
        nc.scalar.activation(st[n][:, 1, :], st[n][:, 1, :], AF.Sqrt,
                             bias=eps_sb[:, :], scale=1.0)
        nc.vector.reciprocal(st[n][:, 1, :], st[n][:, 1, :])
    for n in range(2):
        nsl = slice(n * 512, (n + 1) * 512)
        meanB = pln.tile([128, 512], F32, tag="meanB", bufs=2, name="meanB")
        rstdB = pln.tile([128, 512], F32, tag="rstdB", bufs=2, name="rstdB")
        nc.gpsimd.partition_broadcast(meanB[:, :], st[n][:, 0, :])
        nc.gpsimd.partition_broadcast(rstdB[:, :], st[n][:, 1, :])
        for k in range(KT):
            tmp = pln.tile([128, 512], F32, tag="lnt", bufs=3, name="tmp")
            eng = nc.gpsimd if k % 3 == 2 else nc.vector
            eng.tensor_sub(tmp[:, :], src[:, k, nsl], meanB[:, :])
            eng.tensor_mul(tmp[:, :], tmp[:, :], rstdB[:, :])
            nc.scalar.activation(
                dst[:, k, nsl], tmp[:, :], AF.Identity,
                bias=ada_pp[:, shift_c, k:k + 1],
                scale=ada_pp[:, scale_c, k:k + 1],
            )


def _truncate_out(tc, nc, out_dram):
    with tc.tile_pool(name="ptrunc", bufs=1) as p:
        z = p.tile([128, D], F32, name="z")
        nc.vector.memset(z[:, :], 0.0)
        for tt in range(NT // 128):
            nc.sync.dma_start(out_dram[tt * 128:(tt + 1) * 128, :], z[:, :])


def _body(tc, ins, out_dram):
    nc = tc.nc
    phase_limit = float(os.environ.get("BASS_PHASES", "6"))
    ctx = ExitStack()
    with ctx:
        dram = ctx.enter_context(tc.tile_pool(name="dram", bufs=1, space="DRAM"))
        ada_dr = dram.tile([6 * D], F32)
        qk_dr = dram.tile([2 * D, NT], F8)
        attn_dr = dram.tile([D, NT], F8)

        pers = ctx.enter_context(tc.tile_pool(name="pers", bufs=1))
        ident = pers.tile([128, 128], F32)
        make_identity(nc, ident[:, :])
        identR = pers.tile([128, 128], F32R)
        nc.vector.tensor_copy(identR[:, :], ident[:, :])
        ones_f = pers.tile([128, 1], F32)
        nc.vector.memset(ones_f[:, :], 1.0)
        ones_r = pers.tile([128, 1], F32R)
        nc.vector.tensor_copy(ones_r[:, :], ones_f[:, :])
        eps_sb = pers.tile([1, 1], F32)
        nc.vector.memset(eps_sb[:, :], EPS)
        mln16 = pers.tile([128, 1], F32)
        nc.vector.memset(mln16[:, :], -LN16)
        t_pp = pers.tile([128, KT], F32)
        nc.sync.dma_start(t_pp[:, :], ins["t_emb"].rearrange("(k p) -> p k", p=128))
        t_sl = pers.tile([128, KT], F32R)
        nc.scalar.activation(t_sl[:, :], t_pp[:, :], AF.Silu)

        bqk_pp = pers.tile([128, MQK], F32)
        bv_row = pers.tile([1, H, HD], F32)
        bvB = pers.tile([128, H, HD], F32)
        bproj_pp = pers.tile([128, KT], F32)
        bfc1_pp = pers.tile([128, MH], F32)
        bfc2_pp = pers.tile([128, KT], F32)
        bada_pp = pers.tile([128, 6, KT], F32)
        ada_pp = pers.tile([128, 6, KT], F32)

        def emit_bias_loads():
            nc.sync.dma_start(
                bqk_pp[:, :],
                ins["b_qkv"][0:2 * D].rearrange("(m p) -> p m", p=128))
            nc.sync.dma_start(
                bv_row[:, :, :],
                ins["b_qkv"][2 * D:3 * D].rearrange("(a h d) -> a h d",
                                                    a=1, h=H))
            nc.gpsimd.partition_broadcast(bvB[:, :, :], bv_row[:, :, :])
            nc.sync.dma_start(
                bproj_pp[:, :], ins["b_proj"].rearrange("(m p) -> p m", p=128))
            nc.sync.dma_start(
                bfc1_pp[:, :], ins["b_fc1"].rearrange("(m p) -> p m", p=128))
            nc.sync.dma_start(
                bfc2_pp[:, :], ins["b_fc2"].rearrange("(m p) -> p m", p=128))
            nc.sync.dma_start(
                bada_pp[:, :, :],
                ins["b_ada"].rearrange("(c k p) -> p c k", k=KT, p=128))

        xT = pers.tile([128, KT, NT], F32R)   # becomes x2T after residual 1
        # long-lived flat f32 weight stream: all of w_qkv/w_proj/w_fc1/w_fc2
        # flow through this one tag so DMA prefetch crosses phase boundaries
        pwst = ctx.enter_context(tc.tile_pool(name="pwst", bufs=1))

        def load_w_flat(src2d, c0, c1, rows=D):
            """DMA rows [0:rows) x cols [c0:c1) of src2d into a flat
            [128, 3456]-f32 stream tile in (k m) order."""
            wt = pwst.tile([128, 3456], F32, tag="wst", bufs=3, name="wst")
            ncols = c1 - c0
            nc.sync.dma_start(
                wt[:, 0:(rows // 128) * ncols]
                .rearrange("p (k m) -> p k m", m=ncols),
                src2d[0:rows, c0:c1].rearrange("(k p) m -> p k m", p=128),
            )
            return wt

        def emit_ada_chunk(c, pool, pspool, psbufs=2, wbufs=2):
            wada_t = pool.tile([128, KT, 384], F32R, tag="wada", bufs=wbufs,
                               name="wada_t")
            nc.sync.dma_start(
                wada_t[:, :, :],
                ins["w_ada"][:, c * 384:(c + 1) * 384]
                .rearrange("(k p) m -> p k m", p=128).bitcast(F32R),
            )
            pa = pspool.tile([1, 384], F32, tag="psada", bufs=psbufs,
                             name="pa")
            for k in range(KT):
                nc.tensor.matmul(
                    pa[:, :], t_sl[:, k:k + 1], wada_t[:, k, :],
                    start=(k == 0), stop=(k == KT - 1),
                )
            asb = pool.tile([1, 384], F32, tag="asb", bufs=2, name="asb")
            nc.vector.tensor_copy(asb[:, :], pa[:, :])
            nc.gpsimd.dma_start(
                ada_dr[c * 384:(c + 1) * 384].rearrange("(a b) -> a b", a=1),
                asb[0:1, :],
            )

        def emit_ada_gather(c0, c1):
            for c in range(c0, c1):
                nc.sync.dma_start(
                    ada_pp[:, c, :],
                    ada_dr[c * D:(c + 1) * D].rearrange("(k p) -> p k", p=128),
                )
            nc.vector.tensor_add(ada_pp[:, c0:c1, :], ada_pp[:, c0:c1, :],
                                 bada_pp[:, c0:c1, :])

        # ============ phase 1: ada 0-5, x load+transpose, LN1 ================
        es_mod1 = ExitStack()
        pmod1 = es_mod1.enter_context(tc.tile_pool(name="pmod1", bufs=1))
        mod1T = pmod1.tile([128, KT, NT], F8, name="mod1T")

        with tc.tile_pool(name="p1w", bufs=1) as p1w, \
             tc.tile_pool(name="pst", bufs=1) as pst, \
             tc.tile_pool(name="pln", bufs=1) as pln:
            with tc.tile_pool(name="ps_pro", bufs=1, space="PSUM") as ps_pro, \
                 tc.tile_pool(name="pxin", bufs=3) as pxin, \
                 tc.tile_pool(name="ps_tr", bufs=4, space="PSUM") as ps_tr:

                def emit_transpose_block(tt):
                    xin = pxin.tile([128, D], F32R, tag="xin", name="xin")
                    nc.sync.dma_start(
                        xin[:, :],
                        ins["x"][tt * 128:(tt + 1) * 128, :].bitcast(F32R))
                    for kd in range(KT):
                        pt = ps_tr.tile([128, 128], F32, tag="ptr", name="pt")
                        nc.tensor.transpose(
                            _r(pt[:, :]),
                            xin[:, kd * 128:(kd + 1) * 128],
                            identR[:, :],
                        )
                        tsl = slice(tt * 128, (tt + 1) * 128)
                        if kd % 2 == 0:
                            nc.vector.tensor_copy(xT[:, kd, tsl], pt[:, :])
                        else:
                            nc.scalar.copy(xT[:, kd, tsl], pt[:, :])

                for i in range(8):
                    emit_transpose_block(i)
                    if i < 6:
                        emit_ada_chunk(i, p1w, ps_pro)
                    if i == 0:
                        emit_bias_loads()
                emit_ada_gather(0, 2)
                nc.vector.tensor_scalar_add(ada_pp[:, 1, :], ada_pp[:, 1, :],
                                            1.0)

            if phase_limit > 0.6:
                with tc.tile_pool(name="ps_st", bufs=4, space="PSUM") as ps_st:
                    _ln_stats_and_modulate(
                        tc, nc, xT, mod1T, ada_pp, 0, 1, ones_r, eps_sb,
                        pst, pln, ps_st,
                    )

        if phase_limit <= 1:
            es_mod1.close()
            return _truncate_out(tc, nc, out_dram)

        # ============ phase 2: qkv ==========================================
        es_vaug = ExitStack()
        pvaug = es_vaug.enter_context(
            tc.tile_pool(name="pvaug", bufs=1, side="right"))
        es_qkv = ExitStack()
        pqks = es_qkv.enter_context(tc.tile_pool(name="pqks", bufs=1, side="right"))
        qk_st = pqks.tile([128, MQK, 2, 512], F8, name="qk_st")
        # per head: cols 0..72 = v + bv, col 96 = ones (softmax denominator),
        # cols 72..96 zero padding
        v_aug = pvaug.tile([128, NT // 128, H, 97], F8, name="v_aug")
        nc.gpsimd.memset(v_aug[:, :, :, HD:97], 0.0)
        nc.gpsimd.memset(v_aug[:, :, :, 96:97], 1.0)

        with tc.tile_pool(name="p2w", bufs=1) as p2w, \
             tc.tile_pool(name="ps_mm", bufs=2, space="PSUM") as ps_mm, \
             tc.tile_pool(name="ps_mv", bufs=2, space="PSUM") as ps_mv:
            for g in range(6):
                wt = load_w_flat(ins["w_qkv"], g * 384, (g + 1) * 384)
                wqk8 = p2w.tile([128, KT, 384], F8, tag="ws8", bufs=3,
                                name="wqk8")
                _conv8(nc, g, wqk8[:, :, :], wt[:, :])
                for mi in range(3):
                    mo = g * 3 + mi
                    msl = slice(mi * 128, (mi + 1) * 128)
                    pq = ps_mm.tile([128, 2, 512], F32, tag="mm", name="pq")
                    for n in range(2):
                        _dr_group(nc, pq[:, n, :], wqk8, msl, mod1T,
                                  slice(n * 512, (n + 1) * 512))
                    nc.scalar.activation(
                        qk_st[:, mo, :, :], pq[:, :, :], AF.Identity,
                        bias=bqk_pp[:, mo:mo + 1], scale=OS,
                    )
                if g == 2:
                    nc.sync.dma_start(
                        qk_dr[0:D, :].rearrange("(m p) (a n) -> p m a n",
                                                p=128, a=2),
                        qk_st[:, 0:KT, :, :],
                    )
                if g == 5:
                    nc.sync.dma_start(
                        qk_dr[D:2 * D, :].rearrange("(m p) (a n) -> p m a n",
                                                    p=128, a=2),
                        qk_st[:, KT:MQK, :, :],
                    )
            for s in range(3):
                c0 = s * 384
                wt = load_w_flat(ins["w_qkv"], 2 * D + c0, 2 * D + c0 + 384)
                wv8 = p2w.tile([128, KT, 384], F8, tag="wv8", bufs=2,
                               name="wv8")
                _conv8(nc, s + 1, wv8[:, :, :], wt[:, :])
                # merge consecutive full heads into single copies
                pieces = []
                for (h, j0, ln, off) in _col_head_segs(c0, 384):
                    if pieces and j0 == 0 and ln == HD \
                            and pieces[-1][1] == 0 \
                            and pieces[-1][2] % HD == 0 \
                            and pieces[-1][0] + pieces[-1][2] // HD == h:
                        pieces[-1] = (pieces[-1][0], 0, pieces[-1][2] + ln,
                                      pieces[-1][3])
                    else:
                        pieces.append((h, j0, ln, off))
                for tt in range(NT // 128):
                    ttsl = slice(tt * 128, (tt + 1) * 128)
                    pmv = ps_mv.tile([128, 384], F32, tag="mv", bufs=3, name="pmv")
                    for j in range(4):
                        nc.tensor.matmul(
                            pmv[:, :], mod1T[:, 2 * j:2 * j + 2, ttsl],
                            wv8[:, 2 * j:2 * j + 2, :], start=(j == 0),
                            stop=False, perf_mode=DRM, skip_group_check=True,
                        )
                    nc.tensor.matmul(
                        pmv[:, :], mod1T[:, 8, ttsl], wv8[:, 8, :],
                        start=False, stop=True, skip_group_check=True,
                    )
                    for pi, (h, j0, ln, off) in enumerate(pieces):
                        if j0 == 0 and ln % HD == 0:
                            dst = v_aug[:, tt, h:h + ln // HD, 0:HD]
                            bvs = bvB[:, h:h + ln // HD, :]
                        else:
                            dst = v_aug[:, tt, h, j0:j0 + ln]
                            bvs = bvB[:, h, j0:j0 + ln]
                        nc.vector.scalar_tensor_tensor(
                            dst, pmv[:, off:off + ln], OS, bvs,
                            ALU.mult, ALU.add,
                        )
        es_mod1.close()
        es_qkv.close()
        if phase_limit <= 2:
            es_vaug.close()
            return _truncate_out(tc, nc, out_dram)

        # ============ phase 3: attention + fc2 weight prefetch ==============
        def load_w2(m):
            """w_fc2 column block m: 36 k-tiles in two flat pieces."""
            w28 = pwst.tile([128, MH, 128], F8, tag="w28", bufs=3,
                            name="w28")
            wtA = load_w_flat(ins["w_fc2"], m * 128, (m + 1) * 128,
                              rows=27 * 128)
            _conv8(nc, m, w28[:, 0:27, :], wtA[:, 0:27 * 128])
            wtB = pwst.tile([128, 3456], F32, tag="wst", bufs=3,
                            name="wstB")
            nc.sync.dma_start(
                wtB[:, 0:9 * 128].rearrange("p (k m) -> p k m", m=128),
                ins["w_fc2"][27 * 128:36 * 128, m * 128:(m + 1) * 128]
                .rearrange("(k p) m -> p k m", p=128),
            )
            _conv8(nc, m + 1, w28[:, 27:36, :], wtB[:, 0:9 * 128])
            return w28

        w28s = {}
        es_ao = ExitStack()
        pastk = es_ao.enter_context(tc.tile_pool(name="pastk", bufs=2))
        pqdr = es_ao.enter_context(tc.tile_pool(name="pqdr", bufs=1))
        qDR = pqdr.tile([36, 2, H, NT], F8, name="qDR")
        kDR = pqdr.tile([36, 2, H, NT], F8, name="kDR")
        for i in range(2):
            nc.sync.dma_start(
                qDR[:, i, :, :],
                qk_dr[0:D, :].rearrange("(h r) n -> r h n", h=H)
                [36 * i:36 * (i + 1), :, :],
            )
            nc.sync.dma_start(
                kDR[:, i, :, :],
                qk_dr[D:2 * D, :].rearrange("(h r) n -> r h n", h=H)
                [36 * i:36 * (i + 1), :, :],
            )

        with tc.tile_pool(name="p3w", bufs=1) as p3w, \
             tc.tile_pool(name="pexp", bufs=2) as pexp, \
             tc.tile_pool(name="pattn", bufs=2) as pattn, \
             tc.tile_pool(name="ps_s", bufs=2, space="PSUM") as ps_s, \
             tc.tile_pool(name="ps_av", bufs=1, space="PSUM") as ps_av:

            def emit_filler(h):
                # late ada chunks (12) + fc2 m0-1 prefetch spread across
                # the 16 head iterations
                if h < 12:
                    emit_ada_chunk(6 + h, p3w, ps_av, psbufs=2, wbufs=1)
                if h in (7, 11, 15):
                    w28s[(h - 7) // 4] = load_w2((h - 7) // 4)
                if h == 11:
                    emit_ada_gather(2, 6)
                    nc.vector.tensor_scalar_add(ada_pp[:, 4, :],
                                                ada_pp[:, 4, :], 1.0)

            for h in range(H):
                emit_filler(h)
                attn_h = pastk.tile([72, NT], F8, tag="ah", name="attn_h")
                for n in range(2):
                    nsl = slice(n * 512, (n + 1) * 512)
                    exp_hn = pexp.tile([128, NT // 128, 512], F8, tag="exp",
                                       bufs=2, name="exp_hn")
                    for g in range(4):
                        pss = ps_s.tile([128, 2, 512], F32, tag="s",
                                        name="pss")
                        for kk in range(2):
                            kt_i = 2 * g + kk
                            nc.tensor.matmul(
                                pss[:, kk, :],
                                kDR[:, :, h, kt_i * 128:(kt_i + 1) * 128],
                                qDR[:, :, h, nsl],
                                start=True, stop=True, perf_mode=DRM,
                                skip_group_check=True,
                            )
                        nc.scalar.activation(
                            exp_hn[:, 2 * g:2 * g + 2, :], pss[:, :, :],
                            AF.Exp, bias=mln16[:, :], scale=ISC,
                        )
                    pav = ps_av.tile([97, 512], F32, tag="av", bufs=2,
                                     name="pav")
                    for j in range(4):
                        nc.tensor.matmul(
                            pav[:, :], v_aug[:, 2 * j:2 * j + 2, h, 0:97],
                            exp_hn[:, 2 * j:2 * j + 2, :],
                            start=(j == 0), stop=(j == 3), perf_mode=DRM,
                            skip_group_check=True,
                        )
                    recip = pattn.tile([1, 512], F32, tag="recip", bufs=1,
                                       name="recip")
                    nc.vector.reciprocal(recip[:, :], pav[96:97, :])
                    bca = pattn.tile([72, 512], F32, tag="bca", bufs=1, name="bca")
                    nc.gpsimd.partition_broadcast(bca[:, :], recip[:, :])
                    nc.vector.tensor_mul(
                        attn_h[:, nsl], pav[0:72, :], bca[:, :],
                    )
                # head-regroup: write this head's rows to DRAM
                nc.gpsimd.dma_start(
                    attn_dr[h * HD:(h + 1) * HD, :], attn_h[:, :],
                )
        es_vaug.close()
        es_ao.close()
        if phase_limit <= 3:
            return _truncate_out(tc, nc, out_dram)

        # ============ phase 4: proj + residual1 + LN2 ========================
        es_mod2 = ExitStack()
        pmod2 = es_mod2.enter_context(tc.tile_pool(name="pmod2", bufs=1))
        mod2T = pmod2.tile([128, KT, NT], F8, name="mod2T")
        es_at = ExitStack()
        pat = es_at.enter_context(tc.tile_pool(name="pat", bufs=1))
        attn_st = pat.tile([128, KT, NT], F8, name="attn_st")
        nc.sync.dma_start(
            attn_st[:, :, :],
            attn_dr[:, :].rearrange("(k p) n -> p k n", p=128),
        )

        with tc.tile_pool(name="p4w", bufs=1) as p4w, \
             tc.tile_pool(name="pst4", bufs=1) as pst4, \
             tc.tile_pool(name="pln4", bufs=1) as pln4:
            with tc.tile_pool(name="ps_mm2", bufs=3, space="PSUM") as ps_mm2:
                for s in range(3):
                    wt = load_w_flat(ins["w_proj"], s * 384, (s + 1) * 384)
                    wp8 = p4w.tile([128, KT, 384], F8, tag="wp8", bufs=2,
                                   name="wp8")
                    _conv8(nc, s, wp8[:, :, :], wt[:, :])
                    for mi in range(3):
                        mo = s * 3 + mi
                        msl = slice(mi * 128, (mi + 1) * 128)
                        pm2 = ps_mm2.tile([128, 2, 512], F32, tag="mm2",
                                          name="pm2")
                        for n in range(2):
                            _dr_group(nc, pm2[:, n, :], wp8, msl, attn_st,
                                      slice(n * 512, (n + 1) * 512))
                        t_sb = p4w.tile([128, 2, 512], F32, tag="tsb", bufs=2,
                                        name="t_sb")
                        nc.scalar.activation(
                            t_sb[:, :, :], pm2[:, :, :], AF.Identity,
                            bias=bproj_pp[:, mo:mo + 1], scale=OS,
                        )
                        for n in range(2):
                            nsl = slice(n * 512, (n + 1) * 512)
                            nc.vector.scalar_tensor_tensor(
                                xT[:, mo, nsl], t_sb[:, n, :],
                                ada_pp[:, 2, mo:mo + 1],
                                xT[:, mo, nsl], ALU.mult, ALU.add,
                            )

            with tc.tile_pool(name="ps_st2", bufs=4, space="PSUM") as ps_st2:
                _ln_stats_and_modulate(
                    tc, nc, xT, mod2T, ada_pp, 3, 4, ones_r, eps_sb,
                    pst4, pln4, ps_st2,
                )
        es_at.close()
        if phase_limit <= 4:
            es_mod2.close()
            return _truncate_out(tc, nc, out_dram)

        # ============ phase 5a: fc1 =========================================
        es_h = ExitStack()
        ph5 = es_h.enter_context(tc.tile_pool(name="ph5", bufs=1, side="right"))
        hT = ph5.tile([128, MH, 2, 512], F8, name="hT")

        with tc.tile_pool(name="p5aw", bufs=1) as p5aw, \
             tc.tile_pool(name="ps_f1", bufs=3, space="PSUM") as ps_f1:
            for b in range(12):
                wt = load_w_flat(ins["w_fc1"], b * 384, (b + 1) * 384)
                wf18 = p5aw.tile([128, KT, 384], F8, tag="wf8", bufs=3,
                                 name="wf18")
                _conv8(nc, b, wf18[:, :, :], wt[:, :])
                for mi in range(3):
                    mo = b * 3 + mi
                    msl = slice(mi * 128, (mi + 1) * 128)
                    pf1 = ps_f1.tile([128, 2, 512], F32, tag="f1",
                                     name="pf1")
                    for n in range(2):
                        _dr_group(nc, pf1[:, n, :], wf18, msl, mod2T,
                                  slice(n * 512, (n + 1) * 512))
                    nc.scalar.activation(
                        hT[:, mo, :, :], pf1[:, :, :], AF.Gelu_apprx_tanh,
                        bias=bfc1_pp[:, mo:mo + 1], scale=OS,
                    )
        es_mod2.close()

        # ============ phase 5b: fc2 + residual2 + output =====================
        es_o = ExitStack()
        po = es_o.enter_context(tc.tile_pool(name="po", bufs=1))
        o_full = po.tile([128, KT, NT], F32R, name="o_full")

        with tc.tile_pool(name="p5w", bufs=1) as p5w, \
             tc.tile_pool(name="ph", bufs=1) as ph, \
             tc.tile_pool(name="ps_f2", bufs=4, space="PSUM") as ps_f2, \
             tc.tile_pool(name="ps_tro", bufs=2, space="PSUM") as ps_tro:
            for ms in ([0, 1], [2, 3], [4, 5], [6, 7], [8]):
                pms = {}
                for m in ms:
                    if m not in w28s:
                        w28s[m] = load_w2(m)
                    for n in range(2):
                        pms[(m, n)] = ps_f2.tile(
                            [128, 512], F32, tag="f2", bufs=4,
                            name=f"f2_{m}_{n}"
                        )
                for j in range(MH // 2):
                    for m in ms:
                        for n in range(2):
                            nc.tensor.matmul(
                                pms[(m, n)][:, :],
                                w28s[m][:, 2 * j:2 * j + 2, :],
                                hT[:, 2 * j:2 * j + 2, n, :],
                                start=(j == 0), stop=(j == MH // 2 - 1),
                                perf_mode=DRM, skip_group_check=True,
                            )
                for m in ms:
                    for n in range(2):
                        nsl = slice(n * 512, (n + 1) * 512)
                        t2 = p5w.tile([128, 512], F32, tag="tsb", bufs=3,
                                      name="t2")
                        nc.scalar.activation(
                            t2[:, :], pms[(m, n)][:, :], AF.Identity,
                            bias=bfc2_pp[:, m:m + 1], scale=OS,
                        )
                        nc.vector.scalar_tensor_tensor(
                            o_full[:, m, nsl], t2[:, :],
                            ada_pp[:, 5, m:m + 1], xT[:, m, nsl],
                            ALU.mult, ALU.add,
                        )
                    ot = ph.tile([128, NT // 128, 128], F32, tag="ot",
                                 bufs=3, name="ot")
                    for tt in range(NT // 128):
                        pt = ps_tro.tile([128, 128], F32, tag="tro", bufs=2,
                                         name="pt6")
                        nc.tensor.transpose(
                            _r(pt[:, :]),
                            o_full[:, m, tt * 128:(tt + 1) * 128],
                            identR[:, :],
                        )
                        if tt % 2 == 0:
                            nc.vector.tensor_copy(ot[:, tt, :], pt[:, :])
                        else:
                            nc.scalar.copy(ot[:, tt, :], pt[:, :])
                    nc.gpsimd.dma_start(
                        out_dram[:, m * 128:(m + 1) * 128]
                        .rearrange("(tt p) c -> p tt c", p=128),
                        ot[:, :, :],
                    )
        es_o.close()
        es_h.close()


_LOCK = threading.Lock()
_PROG = None


def _get_program():
    global _PROG
    with _LOCK:
        if _PROG is None:
            _PROG = _build_program()
    return _PROG


def _make_in_maps(inputs):
    arrs = {k: np.ascontiguousarray(np.asarray(v, dtype=np.float32))
            for k, v in inputs.items()}
    in_maps = []
    for c in range(NCORES):
        m = {k: v for k, v in arrs.items() if k not in ("x", "t_emb")}
        m["x"] = np.ascontiguousarray(arrs["x"][c])
        m["t_emb"] = np.ascontiguousarray(arrs["t_emb"][c])
        in_maps.append(m)
    return in_maps


def kernel(**inputs):
    nc = _get_program()
    res = run_bass_kernel_spmd(nc, _make_in_maps(inputs), core_ids=list(range(NCORES)))
    return np.stack([r["out"] for r in res.results], axis=0)


def kernel_traced(inputs, **kw):
    """test-harness helper: returns full BassKernelResults with trace."""
    nc = _get_program()
    return run_bass_kernel_spmd(
        nc, _make_in_maps(inputs), core_ids=list(range(NCORES)), trace=True, **kw
    )


# revision 62
# speedup vs baseline: 1.0045x; 1.0045x over previous
"""DiT block kernel for Trainium2 (Bass/Tile), 8-core data parallel.

Shapes (hardcoded from the problem spec):
  x: (8, 1024, 1152), t_emb: (8, 1152)
  w_qkv (1152, 3456), w_proj (1152, 1152), w_fc1 (1152, 4608),
  w_fc2 (4608, 1152), w_ada (1152, 6912) + biases.

Strategy: batch-parallel across 8 cores (one batch element each, no
collectives). Activations live transposed [D on partitions, tokens free].
All large GEMMs run in fp8e4 with DoubleRow packing (contraction pairs
of 128-partition tiles -> 2x PE throughput); weights are scaled x256
into fp8 range on-chip, outputs rescaled 1/256 in the PSUM-drain
activation. Attention scores use a [36,2] DoubleRow split of the 72-dim
head contraction; exp is computed as exp(s*isc - ln16) so softmax
numerators stay inside fp8 range (the denominator ratio is
scale-invariant, and the v-bias rides inside v so softmax-avg adds it
for free). ada projection and LN-stat matmuls stream f32 operands
bitcast to f32r (full PE rate at N>=256, zero conversion cost).
All big weight matrices stream through one long-lived flat
[128, 3456]-f32 pool so DMA prefetch is never write-after-read blocked
by phase-local pools; fp8 conversion happens on DVE/GPSIMD (the
activation engine is the attention-phase bottleneck via exp).
attn head->tile regrouping goes through a DRAM round trip.
"""

import math
import os
import threading
from contextlib import ExitStack

import numpy as np

import concourse.bass as bass
import concourse.mybir as mybir
import concourse.tile as tile
from concourse import bacc
from concourse.bass_utils import run_bass_kernel_spmd
from concourse.masks import make_identity

F32 = mybir.dt.float32
F32R = mybir.dt.float32r
BF16 = mybir.dt.bfloat16
F8 = mybir.dt.float8e4
AF = mybir.ActivationFunctionType
ALU = mybir.AluOpType
DRM = mybir.MatmulPerfMode.DoubleRow

NCORES = 8
D = 1152
NT = 1024          # tokens per core (batch element)
KT = D // 128      # 9 partition-tiles of D
H = 16
HD = 72
HID = 4 * D        # 4608
MQK = (2 * D) // 128   # 18 output tiles for q,k
MH = HID // 128        # 36
EPS = 1e-6
ISC = 1.0 / float(np.sqrt(HD))
WS = 256.0         # weight scale into fp8
OS = 1.0 / WS
LN16 = math.log(16.0)


def _r(ap):
    return ap.bitcast(F32R)


def _head_segs(d0, n):
    """Split logical rows [d0, d0+n) of a [*,128]-tiled stacked tensor into
    (ktile, part0, length, dst_offset) segments within 128-partition tiles."""
    segs = []
    off = 0
    while n > 0:
        kt_i, p0 = divmod(d0, 128)
        ln = min(n, 128 - p0)
        segs.append((kt_i, p0, ln, off))
        d0 += ln
        off += ln
        n -= ln
    return segs


def _col_head_segs(c0, n):
    """Split v-feature columns [c0, c0+n) into (head, col0, len, src_off)
    pieces."""
    segs = []
    off = 0
    while n > 0:
        h, j0 = divmod(c0, HD)
        ln = min(n, HD - j0)
        segs.append((h, j0, ln, off))
        c0 += ln
        off += ln
        n -= ln
    return segs


def _build_program():
    nc = bacc.Bacc(
        "TRN2", target_bir_lowering=False, debug=False, enable_asserts=False
    )
    ins = {}
    ins["x"] = nc.dram_tensor("x", [NT, D], F32, kind="ExternalInput").ap()
    ins["t_emb"] = nc.dram_tensor("t_emb", [D], F32, kind="ExternalInput").ap()
    for name, shape in [
        ("w_qkv", [D, 3 * D]), ("b_qkv", [3 * D]),
        ("w_proj", [D, D]), ("b_proj", [D]),
        ("w_fc1", [D, HID]), ("b_fc1", [HID]),
        ("w_fc2", [HID, D]), ("b_fc2", [D]),
        ("w_ada", [D, 6 * D]), ("b_ada", [6 * D]),
    ]:
        ins[name] = nc.dram_tensor(name, shape, F32, kind="ExternalInput").ap()
    out_dram = nc.dram_tensor("out", [NT, D], F32, kind="ExternalOutput").ap()

    with tile.TileContext(nc) as tc:
        _body(tc, ins, out_dram)
    nc.compile()
    return nc


def _conv8(nc, i, dst, src):
    """f32 -> fp8e4 with x256 scale, DVE/GPSIMD round-robined by index i
    (the activation engine is kept free for exp/gelu/modulate)."""
    if i % 2 == 0:
        nc.vector.tensor_scalar_mul(dst, src, WS)
    else:
        nc.gpsimd.tensor_scalar_mul(dst, src, WS)


def _dr_group(nc, out, w8, msl, act, nsl):
    """out = (w8[:,:,msl]*256).T @ act[:,:,nsl] over KT=9 k-tiles:
    4 DoubleRow pairs + 1 plain fp8 matmul."""
    for j in range(4):
        nc.tensor.matmul(
            out, w8[:, 2 * j:2 * j + 2, msl], act[:, 2 * j:2 * j + 2, nsl],
            start=(j == 0), stop=False, perf_mode=DRM,
            skip_group_check=True,
        )
    nc.tensor.matmul(
        out, w8[:, 8, msl], act[:, 8, nsl], start=False, stop=True,
        skip_group_check=True,
    )


def _ln_stats_and_modulate(tc, nc, src, dst, ada_pp, shift_c, scale_c,
                           ones_r, eps_sb, pst, pln, ps_st):
    """dst[:,k,:] = fp8(((src-mean)*rstd) * ada[scale_c] + ada[shift_c]).
    mean/var over the partition (D) axis per token via ones-vector f32r
    matmuls. Stats for both 512-token halves first, then applies; the
    normalize sub/mul pairs are split DVE/GPSIMD to shorten the chain."""
    ps_x, ps_q, st = {}, {}, {}
    for n in range(2):
        nsl = slice(n * 512, (n + 1) * 512)
        ps_x[n] = ps_st.tile([1, 512], F32, tag="st", name=f"psx{n}")
        ps_q[n] = ps_st.tile([1, 512], F32, tag="st", name=f"psq{n}")
        for k in range(KT):
            sq = pln.tile([128, 512], F32R, tag="sq", bufs=3, name="sq")
            sqe = nc.gpsimd if k % 3 == 1 else nc.vector
            sqe.tensor_mul(sq[:, :], src[:, k, nsl], src[:, k, nsl])
            nc.tensor.matmul(
                ps_x[n][:, :], ones_r[:, :], src[:, k, nsl],
                start=(k == 0), stop=(k == KT - 1), skip_group_check=True,
            )
            nc.tensor.matmul(
                ps_q[n][:, :], ones_r[:, :], sq[:, :],
                start=(k == 0), stop=(k == KT - 1), skip_group_check=True,
            )
    for n in range(2):
        # rows: 0 = mean, 1 = E[x^2] -> rstd
        st[n] = pst.tile([1, 2, 512], F32, tag="lnst", bufs=2, name=f"st{n}")
        nc.vector.tensor_scalar_mul(st[n][:, 0, :], ps_x[n][:, :], 1.0 / D)
        work = pst.tile([1, 512], F32, tag="lnwork", bufs=2, name="work")
        nc.vector.tensor_mul(work[:, :], st[n][:, 0, :], st[n][:, 0, :])
        nc.vector.scalar_tensor_tensor(
            st[n][:, 1, :], ps_q[n][:, :], 1.0 / D, work[:, :],
            ALU.mult, ALU.subtract,
        )
        nc.scalar.activation(st[n][:, 1, :], st[n][:, 1, :], AF.Sqrt,
                             bias=eps_sb[:, :], scale=1.0)
        nc.vector.reciprocal(st[n][:, 1, :], st[n][:, 1, :])
    for n in range(2):
        nsl = slice(n * 512, (n + 1) * 512)
        meanB = pln.tile([128, 512], F32, tag="meanB", bufs=2, name="meanB")
        rstdB = pln.tile([128, 512], F32, tag="rstdB", bufs=2, name="rstdB")
        nc.gpsimd.partition_broadcast(meanB[:, :], st[n][:, 0, :])
        nc.gpsimd.partition_broadcast(rstdB[:, :], st[n][:, 1, :])
        for k in range(KT):
            tmp = pln.tile([128, 512], F32, tag="lnt", bufs=3, name="tmp")
            eng = nc.gpsimd if k % 3 == 2 else nc.vector
            eng.tensor_sub(tmp[:, :], src[:, k, nsl], meanB[:, :])
            eng.tensor_mul(tmp[:, :], tmp[:, :], rstdB[:, :])
            nc.scalar.activation(
                dst[:, k, nsl], tmp[:, :], AF.Identity,
                bias=ada_pp[:, shift_c, k:k + 1],
                scale=ada_pp[:, scale_c, k:k + 1],
            )


def _truncate_out(tc, nc, out_dram):
    with tc.tile_pool(name="ptrunc", bufs=1) as p:
        z = p.tile([128, D], F32, name="z")
        nc.vector.memset(z[:, :], 0.0)
        for tt in range(NT // 128):
            nc.sync.dma_start(out_dram[tt * 128:(tt + 1) * 128, :], z[:, :])


def _body(tc, ins, out_dram):
    nc = tc.nc
    phase_limit = float(os.environ.get("BASS_PHASES", "6"))
    ctx = ExitStack()
    with ctx:
        dram = ctx.enter_context(tc.tile_pool(name="dram", bufs=1, space="DRAM"))
        ada_dr = dram.tile([6 * D], F32)
        qk_dr = dram.tile([2 * D, NT], F8)
        attn_dr = dram.tile([D, NT], F8)

        pers = ctx.enter_context(tc.tile_pool(name="pers", bufs=1))
        ident = pers.tile([128, 128], F32)
        make_identity(nc, ident[:, :])
        identR = pers.tile([128, 128], F32R)
        nc.vector.tensor_copy(identR[:, :], ident[:, :])
        ones_f = pers.tile([128, 1], F32)
        nc.vector.memset(ones_f[:, :], 1.0)
        ones_r = pers.tile([128, 1], F32R)
        nc.vector.tensor_copy(ones_r[:, :], ones_f[:, :])
        eps_sb = pers.tile([1, 1], F32)
        nc.vector.memset(eps_sb[:, :], EPS)
        mln16 = pers.tile([128, 1], F32)
        nc.vector.memset(mln16[:, :], -LN16)
        t_pp = pers.tile([128, KT], F32)
        nc.sync.dma_start(t_pp[:, :], ins["t_emb"].rearrange("(k p) -> p k", p=128))
        t_sl = pers.tile([128, KT], F32R)
        nc.scalar.activation(t_sl[:, :], t_pp[:, :], AF.Silu)

        bqk_pp = pers.tile([128, MQK], F32)
        bv_row = pers.tile([1, H, HD], F32)
        bvB = pers.tile([128, H, HD], F32)
        bproj_pp = pers.tile([128, KT], F32)
        bfc1_pp = pers.tile([128, MH], F32)
        bfc2_pp = pers.tile([128, KT], F32)
        bada_pp = pers.tile([128, 6, KT], F32)
        ada_pp = pers.tile([128, 6, KT], F32)

        def emit_bias_loads():
            nc.sync.dma_start(
                bqk_pp[:, :],
                ins["b_qkv"][0:2 * D].rearrange("(m p) -> p m", p=128))
            nc.sync.dma_start(
                bv_row[:, :, :],
                ins["b_qkv"][2 * D:3 * D].rearrange("(a h d) -> a h d",
                                                    a=1, h=H))
            nc.gpsimd.partition_broadcast(bvB[:, :, :], bv_row[:, :, :])
            nc.sync.dma_start(
                bproj_pp[:, :], ins["b_proj"].rearrange("(m p) -> p m", p=128))
            nc.sync.dma_start(
                bfc1_pp[:, :], ins["b_fc1"].rearrange("(m p) -> p m", p=128))
            nc.sync.dma_start(
                bfc2_pp[:, :], ins["b_fc2"].rearrange("(m p) -> p m", p=128))
            nc.sync.dma_start(
                bada_pp[:, :, :],
                ins["b_ada"].rearrange("(c k p) -> p c k", k=KT, p=128))

        xT = pers.tile([128, KT, NT], F32R)   # becomes x2T after residual 1
        # long-lived flat f32 weight stream: all of w_qkv/w_proj/w_fc1/w_fc2
        # flow through this one tag so DMA prefetch crosses phase boundaries
        pwst = ctx.enter_context(tc.tile_pool(name="pwst", bufs=1))

        def load_w_flat(src2d, c0, c1, rows=D):
            """DMA rows [0:rows) x cols [c0:c1) of src2d into a flat
            [128, 3456]-f32 stream tile in (k m) order."""
            wt = pwst.tile([128, 3456], F32, tag="wst", bufs=3, name="wst")
            ncols = c1 - c0
            nc.sync.dma_start(
                wt[:, 0:(rows // 128) * ncols]
                .rearrange("p (k m) -> p k m", m=ncols),
                src2d[0:rows, c0:c1].rearrange("(k p) m -> p k m", p=128),
            )
            return wt

        def emit_ada_chunk(c, pool, pspool, psbufs=2, wbufs=2):
            wada_t = pool.tile([128, KT, 384], F32R, tag="wada", bufs=wbufs,
                               name="wada_t")
            nc.sync.dma_start(
                wada_t[:, :, :],
                ins["w_ada"][:, c * 384:(c + 1) * 384]
                .rearrange("(k p) m -> p k m", p=128).bitcast(F32R),
            )
            pa = pspool.tile([1, 384], F32, tag="psada", bufs=psbufs,
                             name="pa")
            for k in range(KT):
                nc.tensor.matmul(
                    pa[:, :], t_sl[:, k:k + 1], wada_t[:, k, :],
                    start=(k == 0), stop=(k == KT - 1),
                )
            asb = pool.tile([1, 384], F32, tag="asb", bufs=2, name="asb")
            nc.vector.tensor_copy(asb[:, :], pa[:, :])
            nc.gpsimd.dma_start(
                ada_dr[c * 384:(c + 1) * 384].rearrange("(a b) -> a b", a=1),
                asb[0:1, :],
            )

        def emit_ada_gather(c0, c1):
            for c in range(c0, c1):
                nc.sync.dma_start(
                    ada_pp[:, c, :],
                    ada_dr[c * D:(c + 1) * D].rearrange("(k p) -> p k", p=128),
                )
            nc.vector.tensor_add(ada_pp[:, c0:c1, :], ada_pp[:, c0:c1, :],
                                 bada_pp[:, c0:c1, :])

        # ============ phase 1: ada 0-5, x load+transpose, LN1 ================
        es_mod1 = ExitStack()
        pmod1 = es_mod1.enter_context(tc.tile_pool(name="pmod1", bufs=1))
        mod1T = pmod1.tile([128, KT, NT], F8, name="mod1T")

        with tc.tile_pool(name="p1w", bufs=1) as p1w, \
             tc.tile_pool(name="pst", bufs=1) as pst, \
             tc.tile_pool(name="pln", bufs=1) as pln:
            with tc.tile_pool(name="ps_pro", bufs=1, space="PSUM") as ps_pro, \
                 tc.tile_pool(name="pxin", bufs=3) as pxin, \
                 tc.tile_pool(name="ps_tr", bufs=4, space="PSUM") as ps_tr:

                def emit_transpose_block(tt):
                    xin = pxin.tile([128, D], F32R, tag="xin", name="xin")
                    nc.sync.dma_start(
                        xin[:, :],
                        ins["x"][tt * 128:(tt + 1) * 128, :].bitcast(F32R))
                    for kd in range(KT):
                        pt = ps_tr.tile([128, 128], F32, tag="ptr", name="pt")
                        nc.tensor.transpose(
                            _r(pt[:, :]),
                            xin[:, kd * 128:(kd + 1) * 128],
                            identR[:, :],
                        )
                        tsl = slice(tt * 128, (tt + 1) * 128)
                        if kd % 2 == 0:
                            nc.vector.tensor_copy(xT[:, kd, tsl], pt[:, :])
                        else:
                            nc.scalar.copy(xT[:, kd, tsl], pt[:, :])

                for i in range(8):
                    emit_transpose_block(i)
                    if i < 6:
                        emit_ada_chunk(i, p1w, ps_pro)
                    if i == 0:
                        emit_bias_loads()
                emit_ada_gather(0, 2)
                nc.vector.tensor_scalar_add(ada_pp[:, 1, :], ada_pp[:, 1, :],
                                            1.0)

            if phase_limit > 0.6:
                with tc.tile_pool(name="ps_st", bufs=4, space="PSUM") as ps_st:
                    _ln_stats_and_modulate(
                        tc, nc, xT, mod1T, ada_pp, 0, 1, ones_r, eps_sb,
                        pst, pln, ps_st,
                    )

        if phase_limit <= 1:
            es_mod1.close()
            return _truncate_out(tc, nc, out_dram)

        # ============ phase 2: qkv ==========================================
        es_vaug = ExitStack()
        pvaug = es_vaug.enter_context(
            tc.tile_pool(name="pvaug", bufs=1, side="right"))
        es_qkv = ExitStack()
        pqks = es_qkv.enter_context(tc.tile_pool(name="pqks", bufs=1, side="right"))
        qk_st = pqks.tile([128, MQK, 2, 512], F8, name="qk_st")
        # per head: cols 0..72 = v + bv, col 96 = ones (softmax denominator),
        # cols 72..96 zero padding
        v_aug = pvaug.tile([128, NT // 128, H, 97], F8, name="v_aug")
        nc.gpsimd.memset(v_aug[:, :, :, HD:97], 0.0)
        nc.gpsimd.memset(v_aug[:, :, :, 96:97], 1.0)

        with tc.tile_pool(name="p2w", bufs=1) as p2w, \
             tc.tile_pool(name="ps_mm", bufs=2, space="PSUM") as ps_mm, \
             tc.tile_pool(name="ps_mv", bufs=2, space="PSUM") as ps_mv:
            for g in range(6):
                wt = load_w_flat(ins["w_qkv"], g * 384, (g + 1) * 384)
                wqk8 = p2w.tile([128, KT, 384], F8, tag="ws8", bufs=3,
                                name="wqk8")
                _conv8(nc, g, wqk8[:, :, :], wt[:, :])
                for mi in range(3):
                    mo = g * 3 + mi
                    msl = slice(mi * 128, (mi + 1) * 128)
                    pq = ps_mm.tile([128, 2, 512], F32, tag="mm", name="pq")
                    for n in range(2):
                        _dr_group(nc, pq[:, n, :], wqk8, msl, mod1T,
                                  slice(n * 512, (n + 1) * 512))
                    nc.scalar.activation(
                        qk_st[:, mo, :, :], pq[:, :, :], AF.Identity,
                        bias=bqk_pp[:, mo:mo + 1], scale=OS,
                    )
                if g == 2:
                    nc.sync.dma_start(
                        qk_dr[0:D, :].rearrange("(m p) (a n) -> p m a n",
                                                p=128, a=2),
                        qk_st[:, 0:KT, :, :],
                    )
                if g == 5:
                    nc.sync.dma_start(
                        qk_dr[D:2 * D, :].rearrange("(m p) (a n) -> p m a n",
                                                    p=128, a=2),
                        qk_st[:, KT:MQK, :, :],
                    )
            for s in range(3):
                c0 = s * 384
                wt = load_w_flat(ins["w_qkv"], 2 * D + c0, 2 * D + c0 + 384)
                wv8 = p2w.tile([128, KT, 384], F8, tag="wv8", bufs=2,
                               name="wv8")
                _conv8(nc, s + 1, wv8[:, :, :], wt[:, :])
                # merge consecutive full heads into single copies
                pieces = []
                for (h, j0, ln, off) in _col_head_segs(c0, 384):
                    if pieces and j0 == 0 and ln == HD \
                            and pieces[-1][1] == 0 \
                            and pieces[-1][2] % HD == 0 \
                            and pieces[-1][0] + pieces[-1][2] // HD == h:
                        pieces[-1] = (pieces[-1][0], 0, pieces[-1][2] + ln,
                                      pieces[-1][3])
                    else:
                        pieces.append((h, j0, ln, off))
                for tt in range(NT // 128):
                    ttsl = slice(tt * 128, (tt + 1) * 128)
                    pmv = ps_mv.tile([128, 384], F32, tag="mv", bufs=3, name="pmv")
                    for j in range(4):
                        nc.tensor.matmul(
                            pmv[:, :], mod1T[:, 2 * j:2 * j + 2, ttsl],
                            wv8[:, 2 * j:2 * j + 2, :], start=(j == 0),
                            stop=False, perf_mode=DRM, skip_group_check=True,
                        )
                    nc.tensor.matmul(
                        pmv[:, :], mod1T[:, 8, ttsl], wv8[:, 8, :],
                        start=False, stop=True, skip_group_check=True,
                    )
                    for pi, (h, j0, ln, off) in enumerate(pieces):
                        if j0 == 0 and ln % HD == 0:
                            dst = v_aug[:, tt, h:h + ln // HD, 0:HD]
                            bvs = bvB[:, h:h + ln // HD, :]
                        else:
                            dst = v_aug[:, tt, h, j0:j0 + ln]
                            bvs = bvB[:, h, j0:j0 + ln]
                        nc.vector.scalar_tensor_tensor(
                            dst, pmv[:, off:off + ln], OS, bvs,
                            ALU.mult, ALU.add,
                        )
        es_mod1.close()
        es_qkv.close()
        if phase_limit <= 2:
            es_vaug.close()
            return _truncate_out(tc, nc, out_dram)

        # ============ phase 3: attention + fc2 weight prefetch ==============
        def load_w2(m):
            """w_fc2 column block m: 36 k-tiles in two flat pieces."""
            w28 = pwst.tile([128, MH, 128], F8, tag="w28", bufs=3,
                            name="w28")
            wtA = load_w_flat(ins["w_fc2"], m * 128, (m + 1) * 128,
                              rows=27 * 128)
            _conv8(nc, m, w28[:, 0:27, :], wtA[:, 0:27 * 128])
            wtB = pwst.tile([128, 3456], F32, tag="wst", bufs=3,
                            name="wstB")
            nc.sync.dma_start(
                wtB[:, 0:9 * 128].rearrange("p (k m) -> p k m", m=128),
                ins["w_fc2"][27 * 128:36 * 128, m * 128:(m + 1) * 128]
                .rearrange("(k p) m -> p k m", p=128),
            )
            _conv8(nc, m + 1, w28[:, 27:36, :], wtB[:, 0:9 * 128])
            return w28

        w28s = {}
        es_ao = ExitStack()
        pastk = es_ao.enter_context(tc.tile_pool(name="pastk", bufs=2))
        pqdr = es_ao.enter_context(tc.tile_pool(name="pqdr", bufs=1))
        qDR = pqdr.tile([36, 2, H, NT], F8, name="qDR")
        kDR = pqdr.tile([36, 2, H, NT], F8, name="kDR")
        for i in range(2):
            nc.sync.dma_start(
                qDR[:, i, :, :],
                qk_dr[0:D, :].rearrange("(h r) n -> r h n", h=H)
                [36 * i:36 * (i + 1), :, :],
            )
            nc.sync.dma_start(
                kDR[:, i, :, :],
                qk_dr[D:2 * D, :].rearrange("(h r) n -> r h n", h=H)
                [36 * i:36 * (i + 1), :, :],
            )

        with tc.tile_pool(name="p3w", bufs=1) as p3w, \
             tc.tile_pool(name="pexp", bufs=2) as pexp, \
             tc.tile_pool(name="pattn", bufs=2) as pattn, \
             tc.tile_pool(name="ps_s", bufs=2, space="PSUM") as ps_s, \
             tc.tile_pool(name="ps_av", bufs=1, space="PSUM") as ps_av:

            def emit_filler(h):
                # late ada chunks (12) + fc2 m0-1 prefetch spread across
                # the 16 head iterations
                if h < 12:
                    emit_ada_chunk(6 + h, p3w, ps_av, psbufs=2, wbufs=1)
                if h in (7, 11, 15):
                    w28s[(h - 7) // 4] = load_w2((h - 7) // 4)
                if h == 11:
                    emit_ada_gather(2, 6)
                    nc.vector.tensor_scalar_add(ada_pp[:, 4, :],
                                                ada_pp[:, 4, :], 1.0)

            for h in range(H):
                emit_filler(h)
                attn_h = pastk.tile([72, NT], F8, tag="ah", name="attn_h")
                for n in range(2):
                    nsl = slice(n * 512, (n + 1) * 512)
                    exp_hn = pexp.tile([128, NT // 128, 512], F8, tag="exp",
                                       bufs=2, name="exp_hn")
                    for g in range(4):
                        pss = ps_s.tile([128, 2, 512], F32, tag="s",
                                        name="pss")
                        for kk in range(2):
                            kt_i = 2 * g + kk
                            nc.tensor.matmul(
                                pss[:, kk, :],
                                kDR[:, :, h, kt_i * 128:(kt_i + 1) * 128],
                                qDR[:, :, h, nsl],
                                start=True, stop=True, perf_mode=DRM,
                                skip_group_check=True,
                            )
                        nc.scalar.activation(
                            exp_hn[:, 2 * g:2 * g + 2, :], pss[:, :, :],
                            AF.Exp, bias=mln16[:, :], scale=ISC,
                        )
                    pav = ps_av.tile([97, 512], F32, tag="av", bufs=2,
                                     name="pav")
                    for j in range(4):
                        nc.tensor.matmul(
                            pav[:, :], v_aug[:, 2 * j:2 * j + 2, h, 0:97],
                            exp_hn[:, 2 * j:2 * j + 2, :],
                            start=(j == 0), stop=(j == 3), perf_mode=DRM,
                            skip_group_check=True,
                        )
                    recip = pattn.tile([1, 512], F32, tag="recip", bufs=1,
                                       name="recip")
                    nc.vector.reciprocal(recip[:, :], pav[96:97, :])
                    bca = pattn.tile([72, 512], F32, tag="bca", bufs=1, name="bca")
                    nc.gpsimd.partition_broadcast(bca[:, :], recip[:, :])
                    nc.vector.tensor_mul(
                        attn_h[:, nsl], pav[0:72, :], bca[:, :],
                    )
                # head-regroup: write this head's rows to DRAM
                nc.gpsimd.dma_start(
                    attn_dr[h * HD:(h + 1) * HD, :], attn_h[:, :],
                )
        es_vaug.close()
        es_ao.close()
        if phase_limit <= 3:
            return _truncate_out(tc, nc, out_dram)

        # ============ phase 4: proj + residual1 + LN2 ========================
        es_mod2 = ExitStack()
        pmod2 = es_mod2.enter_context(tc.tile_pool(name="pmod2", bufs=1))
        mod2T = pmod2.tile([128, KT, NT], F8, name="mod2T")
        es_at = ExitStack()
        pat = es_at.enter_context(tc.tile_pool(name="pat", bufs=1))
        attn_st = pat.tile([128, KT, NT], F8, name="attn_st")
        nc.sync.dma_start(
            attn_st[:, :, :],
            attn_dr[:, :].rearrange("(k p) n -> p k n", p=128),
        )

        with tc.tile_pool(name="p4w", bufs=1) as p4w, \
             tc.tile_pool(name="pst4", bufs=1) as pst4, \
             tc.tile_pool(name="pln4", bufs=1) as pln4:
            with tc.tile_pool(name="ps_mm2", bufs=3, space="PSUM") as ps_mm2:
                for s in range(3):
                    wt = load_w_flat(ins["w_proj"], s * 384, (s + 1) * 384)
                    wp8 = p4w.tile([128, KT, 384], F8, tag="wp8", bufs=2,
                                   name="wp8")
                    _conv8(nc, s, wp8[:, :, :], wt[:, :])
                    for mi in range(3):
                        mo = s * 3 + mi
                        msl = slice(mi * 128, (mi + 1) * 128)
                        pm2 = ps_mm2.tile([128, 2, 512], F32, tag="mm2",
                                          name="pm2")
                        for n in range(2):
                            _dr_group(nc, pm2[:, n, :], wp8, msl, attn_st,
                                      slice(n * 512, (n + 1) * 512))
                        t_sb = p4w.tile([128, 2, 512], F32, tag="tsb", bufs=2,
                                        name="t_sb")
                        nc.scalar.activation(
                            t_sb[:, :, :], pm2[:, :, :], AF.Identity,
                            bias=bproj_pp[:, mo:mo + 1], scale=OS,
                        )
                        for n in range(2):
                            nsl = slice(n * 512, (n + 1) * 512)
                            nc.vector.scalar_tensor_tensor(
                                xT[:, mo, nsl], t_sb[:, n, :],
                                ada_pp[:, 2, mo:mo + 1],
                                xT[:, mo, nsl], ALU.mult, ALU.add,
                            )

            with tc.tile_pool(name="ps_st2", bufs=4, space="PSUM") as ps_st2:
                _ln_stats_and_modulate(
                    tc, nc, xT, mod2T, ada_pp, 3, 4, ones_r, eps_sb,
                    pst4, pln4, ps_st2,
                )
        es_at.close()
        if phase_limit <= 4:
            es_mod2.close()
            return _truncate_out(tc, nc, out_dram)

        # ============ phase 5a: fc1 =========================================
        es_h = ExitStack()
        ph5 = es_h.enter_context(tc.tile_pool(name="ph5", bufs=1, side="right"))
        hT = ph5.tile([128, MH, 2, 512], F8, name="hT")

        with tc.tile_pool(name="p5aw", bufs=1) as p5aw, \
             tc.tile_pool(name="ps_f1", bufs=3, space="PSUM") as ps_f1:
            for b in range(12):
                wt = load_w_flat(ins["w_fc1"], b * 384, (b + 1) * 384)
                wf18 = pmod2.tile([128, KT, 384], F8, tag="wf8", bufs=3,
                                 name="wf18")
                _conv8(nc, b, wf18[:, :, :], wt[:, :])
                for mi in range(3):
                    mo = b * 3 + mi
                    msl = slice(mi * 128, (mi + 1) * 128)
                    pf1 = ps_f1.tile([128, 2, 512], F32, tag="f1",
                                     name="pf1")
                    for n in range(2):
                        _dr_group(nc, pf1[:, n, :], wf18, msl, mod2T,
                                  slice(n * 512, (n + 1) * 512))
                    nc.scalar.activation(
                        hT[:, mo, :, :], pf1[:, :, :], AF.Gelu_apprx_tanh,
                        bias=bfc1_pp[:, mo:mo + 1], scale=OS,
                    )
        es_mod2.close()

        # ============ phase 5b: fc2 + residual2 + output =====================
        es_o = ExitStack()
        po = es_o.enter_context(tc.tile_pool(name="po", bufs=1))
        o_full = po.tile([128, KT, NT], F32R, name="o_full")

        with tc.tile_pool(name="p5w", bufs=1) as p5w, \
             tc.tile_pool(name="ph", bufs=1) as ph, \
             tc.tile_pool(name="ps_f2", bufs=4, space="PSUM") as ps_f2, \
             tc.tile_pool(name="ps_tro", bufs=2, space="PSUM") as ps_tro:
            for ms in ([0, 1], [2, 3], [4, 5], [6, 7], [8]):
                pms = {}
                for m in ms:
                    if m not in w28s:
                        w28s[m] = load_w2(m)
                    for n in range(2):
                        pms[(m, n)] = ps_f2.tile(
                            [128, 512], F32, tag="f2", bufs=4,
                            name=f"f2_{m}_{n}"
                        )
                for j in range(MH // 2):
                    for m in ms:
                        for n in range(2):
                            nc.tensor.matmul(
                                pms[(m, n)][:, :],
                                w28s[m][:, 2 * j:2 * j + 2, :],
                                hT[:, 2 * j:2 * j + 2, n, :],
                                start=(j == 0), stop=(j == MH // 2 - 1),
                                perf_mode=DRM, skip_group_check=True,
                            )
                for m in ms:
                    for n in range(2):
                        nsl = slice(n * 512, (n + 1) * 512)
                        t2 = p5w.tile([128, 512], F32, tag="tsb", bufs=3,
                                      name="t2")
                        nc.scalar.activation(
                            t2[:, :], pms[(m, n)][:, :], AF.Identity,
                            bias=bfc2_pp[:, m:m + 1], scale=OS,
                        )
                        nc.vector.scalar_tensor_tensor(
                            o_full[:, m, nsl], t2[:, :],
                            ada_pp[:, 5, m:m + 1], xT[:, m, nsl],
                            ALU.mult, ALU.add,
                        )
                    ot = ph.tile([128, NT // 128, 128], F32, tag="ot",
                                 bufs=3, name="ot")
                    for tt in range(NT // 128):
                        pt = ps_tro.tile([128, 128], F32, tag="tro", bufs=2,
                                         name="pt6")
                        nc.tensor.transpose(
                            _r(pt[:, :]),
                            o_full[:, m, tt * 128:(tt + 1) * 128],
                            identR[:, :],
                        )
                        if tt % 2 == 0:
                            nc.vector.tensor_copy(ot[:, tt, :], pt[:, :])
                        else:
                            nc.scalar.copy(ot[:, tt, :], pt[:, :])
                    nc.gpsimd.dma_start(
                        out_dram[:, m * 128:(m + 1) * 128]
                        .rearrange("(tt p) c -> p tt c", p=128),
                        ot[:, :, :],
                    )
        es_o.close()
        es_h.close()


_LOCK = threading.Lock()
_PROG = None


def _get_program():
    global _PROG
    with _LOCK:
        if _PROG is None:
            _PROG = _build_program()
    return _PROG


def _make_in_maps(inputs):
    arrs = {k: np.ascontiguousarray(np.asarray(v, dtype=np.float32))
            for k, v in inputs.items()}
    in_maps = []
    for c in range(NCORES):
        m = {k: v for k, v in arrs.items() if k not in ("x", "t_emb")}
        m["x"] = np.ascontiguousarray(arrs["x"][c])
        m["t_emb"] = np.ascontiguousarray(arrs["t_emb"][c])
        in_maps.append(m)
    return in_maps


def kernel(**inputs):
    nc = _get_program()
    res = run_bass_kernel_spmd(nc, _make_in_maps(inputs), core_ids=list(range(NCORES)))
    return np.stack([r["out"] for r in res.results], axis=0)


def kernel_traced(inputs, **kw):
    """test-harness helper: returns full BassKernelResults with trace."""
    nc = _get_program()
    return run_bass_kernel_spmd(
        nc, _make_in_maps(inputs), core_ids=list(range(NCORES)), trace=True, **kw
    )
